# revision 13
# baseline (speedup 1.0000x reference)
"""GATv2 layer kernel for Trainium2, sharded across 8 NeuronCores.

Computation (reference):
    Wh = h @ W.T                       [N, F]
    s1 = Wh @ a1, s2 = Wh @ a2         [N]
    e  = leaky_relu(s1[:,None] + s2[None,:], 0.2)
    attention = softmax(e * adj, dim=1)
    out = attention @ Wh               [N, F]

Sharding: rows (destination nodes) split across 8 cores, 1024 rows each.
Each core gets its adj row-block (transposed, bf16) plus replicated h/W/a.

Key identity: exp(leaky(x)) = max(exp(x), exp(0.2x)), and with x = s1_i+s2_j
both exponentials are rank-1.  Rescaling row i by 1/u_i (u = exp(s1), cancels
in softmax) gives

    P'_ij = adj_ij * max(v_j, r_i*q_j) + (1-adj_ij) * w_i
    v = exp(s2), q = exp(0.2*s2), r = exp(-0.8*s1), w = 1/u

so the dense transcendental chain collapses to ONE DVE tensor_scalar
(mult+max) plus the mask mult.  The (1-adj)*w masked part is exact at
finalize:  numden = (acc_A + S_D) + u*acc_B - acc_C  where
acc_B = sum_D (adj*T2)^T Whext, acc_C = sum_D adjT^T Whext, S_D = col-sums
of Whext over DVE-path chunks (u*w = 1 cancels exactly; S_D is folded into
acc_A by a rank-1 matmul mid-stream).

To balance engines, NACT of the 64 column-chunks instead run the direct
(unrescaled) Act-engine path Prelu -> mask -> Exp into accumulator acc_A.
Exp runs on chunk pairs to amortize the ScalarE fixed cost.

PSUM discipline: all 8 banks hold the persistent A/B/C accumulator regions
during the main loop, so every setup matmul (whext etc.) completes before
chunk 0.  start=True clears has_written for a WHOLE bank, so only the
chronologically-first matmul per bank sets it.
"""
import sys

for _p in ("/opt/trn_rl_repo", "/root/.axon_site/_ro/trn_rl_repo"):
    if _p not in sys.path:
        sys.path.insert(0, _p)

import numpy as np
import ml_dtypes
from contextlib import ExitStack

from concourse import bacc, tile, mybir
from concourse.bass_utils import run_bass_kernel_spmd

f32 = mybir.dt.float32
bf16 = mybir.dt.bfloat16
AL = mybir.AluOpType
AF = mybir.ActivationFunctionType

N = 8192
F = 128
NCORES = 8
RPC = N // NCORES          # rows per core = 1024
RT = RPC // 128            # row tiles per core = 8
NCI = N // 128             # column chunks = 64
NACT = 32                  # chunks routed to the Act (Prelu/Exp) path
NEG_SLOPE = 0.2

_CACHE = {}


def _is_act_chunk(ci):
    return (((ci + 1) * NACT) // NCI) > ((ci * NACT) // NCI)


def _build():
    nc = bacc.Bacc("TRN2", target_bir_lowering=False)

    adj_ext = nc.declare_dram_parameter("adjT", [N, RPC], bf16, isOutput=False)
    hT_ext = nc.declare_dram_parameter("hT", [F, N], bf16, isOutput=False)
    hTloc_ext = nc.declare_dram_parameter("hT_loc", [F, RPC], bf16, isOutput=False)
    wt_ext = nc.declare_dram_parameter("wt", [F, F], f32, isOutput=False)  # W^T
    w_ext = nc.declare_dram_parameter("w", [F, F], f32, isOutput=False)    # W
    a1_ext = nc.declare_dram_parameter("a1", [F, 1], f32, isOutput=False)
    a2_ext = nc.declare_dram_parameter("a2", [F, 1], f32, isOutput=False)
    out_ext = nc.declare_dram_parameter("out", [RPC, F], f32, isOutput=True)

    act_cis = [ci for ci in range(NCI) if _is_act_chunk(ci)]
    dve_cis = [ci for ci in range(NCI) if not _is_act_chunk(ci)]
    last_act = act_cis[-1] if act_cis else -1
    last_dve = dve_cis[-1] if dve_cis else -1
    # pair up act chunks for batched Exp
    exp_pair = {}          # ci -> (pair_slot, partner_first_ci or None)
    for idx, ci in enumerate(act_cis):
        if idx % 2 == 0:
            exp_pair[ci] = (0, None)
        else:
            exp_pair[ci] = (1, act_cis[idx - 1])

    with tile.TileContext(nc) as tc, ExitStack() as ctx:
        const = ctx.enter_context(tc.tile_pool(name="const", bufs=1))
        setup = ctx.enter_context(tc.tile_pool(name="setup", bufs=3))
        psum = ctx.enter_context(tc.tile_pool(name="psum", bufs=8, space="PSUM"))
        adj_pool = ctx.enter_context(tc.tile_pool(name="adjp", bufs=12))
        work = ctx.enter_context(tc.tile_pool(name="work", bufs=10))
        pexp = ctx.enter_context(tc.tile_pool(name="pexp", bufs=4))
        outp = ctx.enter_context(tc.tile_pool(name="outp", bufs=4))

        # ---- phase 0: DMAs first (input streams start flowing early) ----
        wt_sb = const.tile([F, F], f32)
        nc.sync.dma_start(out=wt_sb, in_=wt_ext[:, :])
        w_sb = const.tile([F, F], f32)
        nc.sync.dma_start(out=w_sb, in_=w_ext[:, :])
        a1_sb = const.tile([F, 1], f32)
        nc.sync.dma_start(out=a1_sb, in_=a1_ext[:, :])
        a2_sb = const.tile([F, 1], f32)
        nc.sync.dma_start(out=a2_sb, in_=a2_ext[:, :])
        hTloc_sb = const.tile([F, RPC], bf16)
        nc.sync.dma_start(out=hTloc_sb, in_=hTloc_ext[:, :])

        adj_sb = {}
        def fetch_adj(ci):
            t = adj_pool.tile([128, RPC], bf16, tag="adjT", name=f"adjT{ci}")
            nc.sync.dma_start(out=t, in_=adj_ext[128 * ci:128 * ci + 128, :])
            adj_sb[ci] = t
        for ci in range(6):
            fetch_adj(ci)

        hTc_sb = {}
        def fetch_hT(k):
            t = setup.tile([128, 1024], bf16, tag="hTc", name=f"hTc{k}")
            nc.sync.dma_start(out=t, in_=hT_ext[:, 1024 * k:1024 * k + 1024])
            hTc_sb[k] = t
        fetch_hT(0)
        fetch_hT(1)

        ones_row = const.tile([1, 128], f32)
        nc.vector.memset(ones_row, 1.0)
        ones_col_bf = const.tile([128, 1], bf16)
        nc.vector.memset(ones_col_bf, 1.0)

        # persistent tensors
        whext_t = [const.tile([128, 8, F + 1], bf16, name=f"whext{_}")
                   for _ in range(8)]
        sj_cols = const.tile([128, NCI], f32)      # s2, column layout
        v_cols = const.tile([128, NCI], f32)       # exp(s2)
        q_cols = const.tile([128, NCI], f32)       # exp(0.2*s2)
        u_cols = const.tile([128, RT], f32)        # exp(s1) own rows, col layout
        sibc = const.tile([128, RPC], f32)         # s1 own rows, bcast over parts
        rbc = const.tile([128, RPC], bf16)         # exp(-0.8*s1) bcast
        sd_row = const.tile([1, F + 1], f32)
        for jj in range(8):
            nc.vector.memset(whext_t[jj][:, :, F:F + 1], 1.0)

        # w1 = W^T a1, w2 = W^T a2 ; wt2 = [W^T | w2] in bf16
        ps_w = psum.tile([128, 512], f32, tag="acc")
        nc.tensor.matmul(ps_w[:, 0:1], lhsT=w_sb, rhs=a1_sb, start=True, stop=True)
        nc.tensor.matmul(ps_w[:, 1:2], lhsT=w_sb, rhs=a2_sb, start=True, stop=True)
        w1_bf = const.tile([128, 1], bf16)
        nc.vector.tensor_copy(out=w1_bf, in_=ps_w[:, 0:1])
        wt2_sb = const.tile([F, F + 1], bf16)
        nc.vector.tensor_copy(out=wt2_sb[:, 0:F], in_=wt_sb)
        nc.vector.tensor_copy(out=wt2_sb[:, F:F + 1], in_=ps_w[:, 1:2])

        # s1 own rows -> free layout -> broadcast across partitions
        si_sb = const.tile([1, RPC], f32)
        for kk in range(RPC // 512):
            ps_si = psum.tile([128, 512], f32, tag="acc", name=f"psi{kk}")
            nc.tensor.matmul(ps_si[0:1, 0:512], lhsT=w1_bf,
                             rhs=hTloc_sb[:, 512 * kk:512 * kk + 512],
                             start=True, stop=True)
            nc.vector.tensor_copy(out=si_sb[0:1, 512 * kk:512 * kk + 512],
                                  in_=ps_si[0:1, 0:512])
        # exp(-0.8*s1) in free layout
        ri_sb = const.tile([1, RPC], f32)
        nc.scalar.activation(out=ri_sb, in_=si_sb, func=AF.Exp, scale=-0.8)
        for kk in range(RPC // 512):
            ps_sib = psum.tile([128, 512], f32, tag="acc", name=f"psib{kk}")
            nc.tensor.matmul(ps_sib[:, 0:512], lhsT=ones_row,
                             rhs=si_sb[0:1, 512 * kk:512 * kk + 512],
                             start=True, stop=True)
            nc.scalar.copy(out=sibc[:, 512 * kk:512 * kk + 512],
                           in_=ps_sib[:, 0:512])
            ps_rb = psum.tile([128, 512], f32, tag="acc", name=f"psrb{kk}")
            nc.tensor.matmul(ps_rb[:, 0:512], lhsT=ones_row,
                             rhs=ri_sb[0:1, 512 * kk:512 * kk + 512],
                             start=True, stop=True)
            nc.vector.tensor_copy(out=rbc[:, 512 * kk:512 * kk + 512],
                                  in_=ps_rb[:, 0:512])

        # u_cols: exp(s1) for own rows in column layout [128, RT]
        ps_u = psum.tile([128, 8], f32, tag="acc", name="psu")
        for t in range(RT):
            nc.tensor.matmul(ps_u[:, t:t + 1],
                             lhsT=hTloc_sb[:, 128 * t:128 * t + 128],
                             rhs=w1_bf, start=True, stop=True)
        nc.scalar.activation(out=u_cols, in_=ps_u[:, 0:8], func=AF.Exp)

        # ---- phase 1: whext stream (all PSUM-consuming setup) ----
        # 3 chunks share one PSUM tile so the psum->sbuf casts batch; cast
        # work alternates Vector/Scalar to halve the setup critical path.
        sd_ps = psum.tile([1, F + 1], f32, tag="acc", name="sdps")
        for k in range(8):
            if k + 2 < 8:
                fetch_hT(k + 2)
            hTc = hTc_sb[k]
            for gi, m0 in enumerate((0, 3, 6)):
                g = min(3, 8 - m0)
                ps2 = psum.tile([128, 3, F + 1], f32, tag="acc",
                                name=f"pwh{k}_{m0}")
                for j in range(g):
                    m = m0 + j
                    nc.tensor.matmul(ps2[:, j, 0:F + 1],
                                     lhsT=hTc[:, 128 * m:128 * m + 128],
                                     rhs=wt2_sb, start=True, stop=True)
                if (3 * k + gi) % 2 == 0:
                    nc.vector.tensor_copy(out=whext_t[k][:, m0:m0 + g, 0:F],
                                          in_=ps2[:, 0:g, 0:F])
                else:
                    nc.scalar.copy(out=whext_t[k][:, m0:m0 + g, 0:F],
                                   in_=ps2[:, 0:g, 0:F])
                nc.vector.tensor_copy(out=sj_cols[:, 8 * k + m0:8 * k + m0 + g],
                                      in_=ps2[:, 0:g, F])
            nc.scalar.activation(out=v_cols[:, 8 * k:8 * k + 8],
                                 in_=sj_cols[:, 8 * k:8 * k + 8], func=AF.Exp)
            nc.scalar.activation(out=q_cols[:, 8 * k:8 * k + 8],
                                 in_=sj_cols[:, 8 * k:8 * k + 8], func=AF.Exp,
                                 scale=0.2)
            for ci in range(8 * k, 8 * k + 8):
                if not _is_act_chunk(ci):
                    nc.tensor.matmul(
                        sd_ps[0:1, 0:F + 1], lhsT=ones_col_bf,
                        rhs=whext_t[k][:, ci % 8, :],
                        start=(ci == dve_cis[0]), stop=(ci == dve_cis[-1]),
                        skip_group_check=True)
        nc.vector.tensor_copy(out=sd_row, in_=sd_ps[0:1, :])

        # ---- persistent accumulators: 8 PSUM banks, one per row tile ----
        # bank t layout: acc_A at [0:129], acc_B at [136:265], acc_C at [272:401]
        acc_banks = [psum.tile([128, 512], f32, tag="acc", name=f"accb{b}")
                     for b in range(RT)]
        acc_A = [acc_banks[t][:, 0:F + 1] for t in range(RT)]
        acc_B = [acc_banks[t][:, 136:136 + F + 1] for t in range(RT)]
        acc_C = [acc_banks[t][:, 272:272 + F + 1] for t in range(RT)]

        # ---- phase 2: main loop over 64 column chunks ----
        pair_T = [None]
        for ci in range(NCI):
            k, m = ci // 8, ci % 8
            if ci + 6 < NCI:
                fetch_adj(ci + 6)
            adjT = adj_sb[ci]
            wh_rhs = whext_t[k][:, m, :]
            if _is_act_chunk(ci):
                # Act path: P = exp(leaky(s1+s2) * adj), unrescaled
                slot, partner = exp_pair[ci]
                if slot == 0:
                    pair_T[0] = pexp.tile([128, 2 * RPC], bf16, tag="Tp",
                                          name=f"Tp{ci}")
                L = work.tile([128, RPC], bf16, tag="L", name=f"L{ci}")
                nc.scalar.activation(out=L, in_=sibc, func=AF.Prelu,
                                     bias=sj_cols[:, ci:ci + 1],
                                     alpha=NEG_SLOPE)
                nc.vector.tensor_tensor(
                    out=pair_T[0][:, slot * RPC:slot * RPC + RPC],
                    in0=L, in1=adjT, op=AL.mult)
                mm_list = []
                if slot == 1 or ci == act_cis[-1]:
                    P = pexp.tile([128, (slot + 1) * RPC], bf16, tag="Pp",
                                  name=f"P{ci}")
                    nc.scalar.activation(out=P, in_=pair_T[0][:, 0:(slot + 1) * RPC],
                                         func=AF.Exp)
                    if partner is not None:
                        mm_list.append((partner, P, 0))
                    mm_list.append((ci, P, slot))
                for mm_ci, P, sl in mm_list:
                    kk, mm = mm_ci // 8, mm_ci % 8
                    for t in range(RT):
                        nc.tensor.matmul(
                            acc_A[t],
                            lhsT=P[:, sl * RPC + 128 * t:sl * RPC + 128 * t + 128],
                            rhs=whext_t[kk][:, mm, :],
                            start=(mm_ci == 0), stop=(mm_ci == last_act),
                            skip_group_check=True)
            else:
                # DVE path: T2 = max(r_i q_j, v_j); M = T2 * adj  (rescaled)
                T2 = work.tile([128, RPC], bf16, tag="T2", name=f"T2_{ci}")
                nc.vector.tensor_scalar(
                    out=T2, in0=rbc,
                    scalar1=q_cols[:, ci:ci + 1],
                    scalar2=v_cols[:, ci:ci + 1],
                    op0=AL.mult, op1=AL.max)
                M = work.tile([128, RPC], bf16, tag="M", name=f"M{ci}")
                nc.vector.tensor_tensor(out=M, in0=T2, in1=adjT, op=AL.mult)
                for t in range(RT):
                    nc.tensor.matmul(
                        acc_B[t], lhsT=M[:, 128 * t:128 * t + 128],
                        rhs=wh_rhs,
                        start=(ci == 0), stop=(ci == last_dve),
                        skip_group_check=True)
                for t in range(RT):
                    nc.tensor.matmul(
                        acc_C[t], lhsT=adjT[:, 128 * t:128 * t + 128],
                        rhs=wh_rhs,
                        start=False, stop=(ci == last_dve),
                        skip_group_check=True)
            if ci == 40:
                # fold S_D into acc_A as a rank-1 broadcast add; must come
                # after chunk 0 (bank-wide has_written clear) and before the
                # accumulation groups stop.
                for t in range(RT):
                    nc.tensor.matmul(acc_A[t], lhsT=ones_row, rhs=sd_row,
                                     start=False, stop=False,
                                     skip_group_check=True)

        # ---- finalize: numden = (acc_A + S_D) + u*acc_B - acc_C ----
        for t in range(RT):
            ub = outp.tile([128, F + 1], f32, tag="ub", name=f"ub{t}")
            nc.scalar.activation(out=ub, in_=acc_B[t], func=AF.Copy,
                                 scale=u_cols[:, t:t + 1])
            z2 = outp.tile([128, F + 1], f32, tag="z2", name=f"z2{t}")
            nc.vector.tensor_tensor(out=z2, in0=ub, in1=acc_A[t], op=AL.add)
            z3 = outp.tile([128, F + 1], f32, tag="z3", name=f"z3{t}")
            nc.vector.tensor_tensor(out=z3, in0=z2, in1=acc_C[t],
                                    op=AL.subtract)
            rinv = outp.tile([128, 1], f32, tag="rinv", name=f"rinv{t}")
            nc.vector.reciprocal(rinv, z3[:, F:F + 1])
            o_t = outp.tile([128, F], f32, tag="o", name=f"o{t}")
            nc.vector.tensor_scalar(out=o_t, in0=z3[:, 0:F],
                                    scalar1=rinv[:, 0:1], scalar2=None,
                                    op0=AL.mult)
            nc.sync.dma_start(out=out_ext[128 * t:128 * t + 128, :], in_=o_t)

    nc.compile()
    return nc


def _get_nc():
    if "nc" not in _CACHE:
        _CACHE["nc"] = _build()
    return _CACHE["nc"]


def kernel(h, adj, W, a, _trace=False, _trace_kwargs=None):
    h = np.asarray(h, dtype=np.float32)
    adj = np.asarray(adj, dtype=np.float32)
    W = np.asarray(W, dtype=np.float32)
    a = np.asarray(a, dtype=np.float32)

    wt = np.ascontiguousarray(W.T)                    # [fi, fo]
    a1c = np.ascontiguousarray(a[0, :F].reshape(F, 1))
    a2c = np.ascontiguousarray(a[0, F:].reshape(F, 1))
    hT_bf = np.ascontiguousarray(h.T.astype(ml_dtypes.bfloat16))  # [fi, n]
    adjT_bf = adj.astype(ml_dtypes.bfloat16).T        # 0/1 values: lossless

    nc = _get_nc()
    in_maps = []
    for c in range(NCORES):
        r0 = c * RPC
        in_maps.append({
            "adjT": np.ascontiguousarray(adjT_bf[:, r0:r0 + RPC]),
            "hT": hT_bf,
            "hT_loc": np.ascontiguousarray(hT_bf[:, r0:r0 + RPC]),
            "wt": wt,
            "w": W,
            "a1": a1c,
            "a2": a2c,
        })
    kw = {}
    if _trace:
        kw["trace"] = True
        kw.update(_trace_kwargs or {})
    res = run_bass_kernel_spmd(nc, in_maps, core_ids=list(range(NCORES)), **kw)
    out = np.concatenate([res.results[c]["out"] for c in range(NCORES)], axis=0)
    if _trace:
        return out, res
    return out
